# revision 14
# baseline (speedup 1.0000x reference)
"""MultiHeadAttention kernel for 8 trn2 NeuronCores (Bass/Tile).

Problem: B=2, S=2048, E=1024, H=16, D=64 (fp32), boolean mask [B,S,S].
  out = softmax(mask((q W_q^T) (k W_k^T)^T / sqrt(D))) (v W_v^T) W_o^T + b_o

Sharding: batch x head-group. Core c (c = 4*g + r) handles batch g and heads
4r..4r+3. Per core:
  - QKV projections for its 4 heads (fp32r matmuls, full PE rate, ~1e-4 err)
  - attention (scores transposed layout [k_tok, q_tok]; ACT exp; DVE mask
    multiply in bf16; PE AV + rowsum matmuls)
  - AllToAll over all 8 cores reshards head-rows -> token-slices; cross-group
    chunks carry duplicated data and are nulled by zero-padded Wo rows
  - O-projection for its 512-token slice (fp32r), + b_o
Host side: pure layout marshalling (transpose/slice/concat), no arithmetic.
"""

import sys

sys.path.insert(0, "/opt/trn_rl_repo")

import numpy as np
import concourse.bass as bass
import concourse.mybir as mybir
from concourse.tile import TileContext
from concourse import bass_utils

F32 = mybir.dt.float32
F32R = mybir.dt.float32r
BF16 = mybir.dt.bfloat16
I32 = mybir.dt.int32
AF = mybir.ActivationFunctionType
ALU = mybir.AluOpType

P = 128
E = 1024
HPC = 4  # heads per core
EC = HPC * 64  # e_out columns per core (256)

# walrus sync-wait limits: split excess waits onto NoOps (see bir_fix notes)
_wait_counter = [0]


def _fix_bir_waits(raw: bytes) -> bytes:
    import orjson

    m = orjson.loads(raw)
    for fn in m["functions"]:
        for blk in fn["blocks"]:
            out = []
            changed = False
            for inst in blk["instructions"]:
                si = inst.get("sync_info") or {}
                waits = si.get("on_wait") or []
                if len(waits) > 1:
                    for w in waits[:-1]:
                        _wait_counter[0] += 1
                        out.append(
                            {
                                "engine": inst["engine"],
                                "ins": [],
                                "name": f"I-waitfix-{_wait_counter[0]}",
                                "opcode": "NoOp",
                                "outs": [],
                                "sync_info": {"on_update": [], "on_wait": [w]},
                            }
                        )
                    si["on_wait"] = waits[-1:]
                    inst["sync_info"] = si
                    changed = True
                out.append(inst)
            if changed:
                blk["instructions"] = out
    return orjson.dumps(m)


def build(S: int = 2048, dbg: bool = False) -> bass.Bass:
    KC = S // 128  # k-chunks
    QBW = S // 4  # q-block width (tokens per a2a chunk / per dest rank)
    NQB = 4
    NW = min(512, QBW)  # matmul moving chunk (attention)
    NS = min(512, S)  # matmul moving chunk (projections)
    assert QBW % NW == 0

    nc = bass.Bass()

    xqT = nc.declare_dram_parameter("xqT", [E, S], F32, isOutput=False)
    xkT = nc.declare_dram_parameter("xkT", [E, S], F32, isOutput=False)
    xvT = nc.declare_dram_parameter("xvT", [E, S], F32, isOutput=False)
    maskT = nc.declare_dram_parameter("maskT", [S, S], I32, isOutput=False)
    WqT = nc.declare_dram_parameter("WqT", [E, EC], F32, isOutput=False)
    WkT = nc.declare_dram_parameter("WkT", [E, EC], F32, isOutput=False)
    WvT = nc.declare_dram_parameter("WvT", [E, EC], F32, isOutput=False)
    WoTz = nc.declare_dram_parameter("WoTz", [2 * E, E], F32, isOutput=False)
    bq = nc.declare_dram_parameter("bq", [EC], F32, isOutput=False)
    bk = nc.declare_dram_parameter("bk", [EC], F32, isOutput=False)
    bv_b = nc.declare_dram_parameter("bv_b", [P, EC], F32, isOutput=False)
    bo_b = nc.declare_dram_parameter("bo_b", [P, E], F32, isOutput=False)
    out = nc.declare_dram_parameter("out", [QBW, E], F32, isOutput=True)
    if dbg:
        dbg_q = nc.declare_dram_parameter("dbg_q", [P, 2 * S], F32, isOutput=True)
        dbg_k = nc.declare_dram_parameter("dbg_k", [P, 2 * S], F32, isOutput=True)
        dbg_v = nc.declare_dram_parameter("dbg_v", [P, (S // P) * EC], F32, isOutput=True)
        dbg_p = nc.declare_dram_parameter("dbg_p", [P, 2 * QBW], F32, isOutput=True)
        dbg_s = nc.declare_dram_parameter("dbg_s", [P, 2 * QBW], F32, isOutput=True)
        dbg_rs = nc.declare_dram_parameter("dbg_rs", [P, QBW], F32, isOutput=True)

    with TileContext(nc) as tc:
        with (
            tc.tile_pool(name="persist", bufs=1) as pp,
            tc.tile_pool(name="dramp", bufs=1, space="DRAM") as dramp,
        ):
            a2a_in = dramp.tile([8 * EC, QBW], F32)
            a2a_out = dramp.tile([8 * EC, QBW], F32)
            # persistent SBUF tensors
            qT_sb = pp.tile([P, 2, S], F32R)  # [:, m, :] = q.T rows 128m..128m+127
            kT_sb = pp.tile([P, 2, S], F32R)
            v_sb = pp.tile([P, KC, EC], BF16)  # [:, t, :] = v rows 128t.., cols=head dims
            ones_sb = pp.tile([P, 64], BF16)
            nc.vector.memset(ones_sb[:], 1.0)
            bq_sb = pp.tile([P, 2], F32)
            bk_sb = pp.tile([P, 2], F32)
            nc.sync.dma_start(bq_sb[:], bq.rearrange("(m p) -> p m", p=P))
            nc.sync.dma_start(bk_sb[:], bk.rearrange("(m p) -> p m", p=P))
            bv_sb = pp.tile([P, EC], BF16)  # bv pre-broadcast over partitions
            nc.gpsimd.dma_start(bv_sb[:], bv_b[:])
            bo_sb = pp.tile([P, E], F32)
            nc.sync.dma_start(bo_sb[:], bo_b[:])

            # ---------------- Phase A: QKV projections ----------------
            with (
                tc.tile_pool(name="wpool", bufs=1) as wp,
                tc.tile_pool(name="xpool", bufs=3) as xp,
                tc.tile_pool(name="psA", bufs=8, space="PSUM") as psA,
            ):
                wq_sb = wp.tile([P, 8, EC], F32R)
                wk_sb = wp.tile([P, 8, EC], F32R)
                wv_sb = wp.tile([P, 8, EC], F32R)
                nc.gpsimd.dma_start(wq_sb[:], WqT.rearrange("(kt p) m -> p kt m", p=P))
                nc.gpsimd.dma_start(wk_sb[:], WkT.rearrange("(kt p) m -> p kt m", p=P))
                nc.gpsimd.dma_start(wv_sb[:], WvT.rearrange("(kt p) m -> p kt m", p=P))

                for which in range(3):
                    xT, w_sb = [(xqT, wq_sb), (xkT, wk_sb), (xvT, wv_sb)][which]
                    nps = (2 * S) // NS if which < 2 else KC // 2
                    pst = [
                        psA.tile([P, 512], F32, name=f"psA_{which}_{i}", tag="psA")
                        for i in range(nps)
                    ]
                    xts = []
                    for kt in range(8):
                        x_t = xp.tile([P, S], F32R, name=f"x_{which}_{kt}", tag="x")
                        nc.gpsimd.dma_start(x_t[:], xT[kt * P : (kt + 1) * P, :])
                        xts.append(x_t)
                        if which < 2:
                            # q.T / k.T: out [256, S]; lhsT = W tile, rhs = x.T
                            for m in range(2):
                                lhsT = w_sb[:, kt, m * P : (m + 1) * P]
                                for n in range(S // NS):
                                    nc.tensor.matmul(
                                        pst[m * (S // NS) + n][:, :NS],
                                        lhsT,
                                        x_t[:, n * NS : (n + 1) * NS],
                                        start=(kt == 0),
                                        stop=(kt == 7),
                                    )
                        else:
                            # v: out [S, 256]; lhsT = x.T tile, rhs = W k-tile
                            for t in range(KC):
                                # two token-chunks share one PSUM bank: the
                                # accumulation group opens on the even chunk
                                # and closes on the odd one (per 2KB region)
                                nc.tensor.matmul(
                                    pst[t // 2][:, (t % 2) * EC : (t % 2 + 1) * EC],
                                    x_t[:, t * P : (t + 1) * P],
                                    w_sb[:, kt, :],
                                    start=(kt == 0 and t % 2 == 0),
                                    stop=(kt == 7 and t % 2 == 1),
                                )
                    # evictions
                    if which == 0:
                        for m in range(2):
                            for n in range(S // NS):
                                nc.vector.tensor_scalar(
                                    qT_sb[:, m, n * NS : (n + 1) * NS],
                                    pst[m * (S // NS) + n][:, :NS],
                                    bq_sb[:, m : m + 1],
                                    0.125,
                                    ALU.add,
                                    ALU.mult,
                                )
                    elif which == 1:
                        for m in range(2):
                            for n in range(S // NS):
                                nc.vector.tensor_scalar(
                                    kT_sb[:, m, n * NS : (n + 1) * NS],
                                    pst[m * (S // NS) + n][:, :NS],
                                    1.0,
                                    bk_sb[:, m : m + 1],
                                    ALU.mult,
                                    ALU.add,
                                )
                    else:
                        for t in range(KC):
                            nc.vector.tensor_tensor(
                                v_sb[:, t, :],
                                pst[t // 2][:, (t % 2) * EC : (t % 2 + 1) * EC],
                                bv_sb[:],
                                ALU.add,
                            )

            # ---------------- Phase B: attention ----------------
            with (
                tc.tile_pool(name="maskpool", bufs=1) as mp,
                tc.tile_pool(name="ppool", bufs=3) as ppl,
                tc.tile_pool(name="epool", bufs=2) as ep,
                tc.tile_pool(name="sps", bufs=2, space="PSUM") as sps,
                tc.tile_pool(name="avps", bufs=2, space="PSUM") as avps,
                tc.tile_pool(name="rsps", bufs=2, space="PSUM") as rsps,
            ):
                maskbf = mp.tile([P, KC, S], BF16)
                for t in range(KC):
                    nc.gpsimd.dma_start(
                        maskbf[:, t, :], maskT[t * P : (t + 1) * P, :]
                    )

                for qb in range(NQB):
                    qsl = slice(qb * QBW, (qb + 1) * QBW)
                    av_t = [
                        avps.tile([P, 512], F32, name=f"av_{qb}_{pair}", tag="av")
                        for pair in range(2)
                    ]
                    rs_t = [
                        rsps.tile([P, 512], F32, name=f"rs_{qb}_{pair}", tag="rs")
                        for pair in range(2)
                    ]
                    for kc in range(KC):
                        for pair in range(2):
                            s_t = sps.tile(
                                [P, 2, 512], F32, name=f"s_{qb}_{kc}_{pair}", tag="s"
                            )
                            ksl = slice(kc * P, (kc + 1) * P)
                            for h in range(2):
                                prt = slice(h * 64, (h + 1) * 64)
                                for n in range(QBW // NW):
                                    nc.tensor.matmul(
                                        s_t[:, h, n * NW : (n + 1) * NW],
                                        kT_sb[prt, pair, ksl],
                                        qT_sb[prt, pair, qb * QBW + n * NW : qb * QBW + (n + 1) * NW],
                                        start=True,
                                        stop=True,
                                    )
                            if dbg and qb == 0 and kc == 0 and pair == 0:
                                dbg_s_sb = ppl.tile([P, 2 * QBW], F32, name="dbg_s_sb", tag="dbgs")
                                nc.vector.tensor_copy(
                                    dbg_s_sb[:].rearrange("p (h n) -> p h n", h=2),
                                    s_t[:, :, :QBW],
                                )
                                nc.sync.dma_start(dbg_s[:], dbg_s_sb[:])
                            p_t = ppl.tile([P, 2 * QBW], BF16, name="p_t", tag="p")
                            nc.scalar.activation(
                                p_t[:].rearrange("p (h n) -> p h n", h=2),
                                s_t[:, :, :QBW],
                                AF.Exp,
                            )
                            for h in range(2):
                                hsl = slice(h * QBW, (h + 1) * QBW)
                                nc.vector.tensor_mul(
                                    p_t[:, hsl], p_t[:, hsl], maskbf[:, kc, qsl]
                                )
                            if dbg and qb == 0 and kc == 0 and pair == 0:
                                nc.gpsimd.dma_start(dbg_p[:], p_t[:])
                            for h in range(2):
                                hsl = slice(h * QBW, (h + 1) * QBW)
                                dsl = slice(pair * P + h * 64, pair * P + (h + 1) * 64)
                                # one has_written group per bank: h0 clears
                                # the bank at kc==0, h1 joins with start=False
                                # (sim's region check mistracks partition-
                                # sliced APs, hence skip_group_check on h1)
                                nc.tensor.matmul(
                                    av_t[pair][h * 64 : (h + 1) * 64, :QBW],
                                    v_sb[:, kc, dsl],
                                    p_t[:, hsl],
                                    start=(kc == 0),
                                    stop=(kc == KC - 1),
                                    skip_group_check=(h == 1),
                                )
                                nc.tensor.matmul(
                                    rs_t[pair][h * 64 : (h + 1) * 64, :QBW],
                                    ones_sb[:],
                                    p_t[:, hsl],
                                    start=(kc == 0),
                                    stop=(kc == KC - 1),
                                    skip_group_check=(h == 1),
                                )
                    if dbg and qb == 0:
                        dbg_rs_sb = ep.tile([P, QBW], F32, name="dbg_rs_sb", tag="dbgrs")
                        nc.vector.tensor_copy(dbg_rs_sb[:], rs_t[0][:, :QBW])
                        nc.sync.dma_start(dbg_rs[:], dbg_rs_sb[:])
                    # epilogue for this q-block
                    for pair in range(2):
                        rb = ep.tile([P, QBW], F32, name="rb", tag="rb")
                        nc.vector.reciprocal(rb[:], rs_t[pair][:, :QBW])
                        av_f = ep.tile([P, QBW], F32, name="av_f", tag="av_f")
                        nc.vector.tensor_mul(av_f[:], av_t[pair][:, :QBW], rb[:])
                        # stage into a2a chunks qb and qb+4 (same data)
                        for chunk in (qb, qb + 4):
                            nc.sync.dma_start(
                                a2a_in[
                                    chunk * 2 * P + pair * P : chunk * 2 * P
                                    + (pair + 1) * P,
                                    :,
                                ],
                                av_f[:],
                            )

            if dbg:
                for m in range(2):
                    dq = ppl_dump = None
                for m in range(2):
                    pass
            if dbg:
                with tc.tile_pool(name="dbgpool", bufs=2) as dp_:
                    for m in range(2):
                        t1 = dp_.tile([P, S], F32, name="t1", tag="t1")
                        nc.vector.tensor_copy(t1[:], qT_sb[:, m, :])
                        nc.sync.dma_start(dbg_q[:, m * S : (m + 1) * S], t1[:])
                        t2 = dp_.tile([P, S], F32, name="t2", tag="t2")
                        nc.vector.tensor_copy(t2[:], kT_sb[:, m, :])
                        nc.sync.dma_start(dbg_k[:, m * S : (m + 1) * S], t2[:])
                    for t in range(S // P):
                        t3 = dp_.tile([P, EC], F32, name="t3", tag="t3")
                        nc.vector.tensor_copy(t3[:], v_sb[:, t, :])
                        nc.sync.dma_start(dbg_v[:, t * EC : (t + 1) * EC], t3[:])
            # ---------------- Phase C: A2A + O-projection ----------------
            nc.gpsimd.collective_compute(
                "AllToAll",
                ALU.bypass,
                ins=[a2a_in[:]],
                outs=[a2a_out[:]],
                replica_groups=[list(range(8))],
            )
            with (
                tc.tile_pool(name="cpool", bufs=1) as cp,
                tc.tile_pool(name="opool", bufs=2) as op,
                tc.tile_pool(name="ops", bufs=2, space="PSUM") as ops,
            ):
                attnT = cp.tile([P, 16, QBW], F32R)
                nc.gpsimd.dma_start(
                    attnT[:], a2a_out.rearrange("(kt p) n -> p kt n", p=P)
                )
                woT_sb = cp.tile([P, 16, E], F32R)
                nc.gpsimd.dma_start(
                    woT_sb[:], WoTz.rearrange("(kt p) n -> p kt n", p=P)
                )
                MT = min(P, QBW)
                for m in range(QBW // MT):
                    o_ps = ops.tile([P, E], F32, name=f"o_{m}", tag="o")
                    for kt in range(16):
                        lhsT = attnT[:, kt, m * MT : (m + 1) * MT]
                        for n in range(2):
                            nc.tensor.matmul(
                                o_ps[:MT, n * 512 : (n + 1) * 512],
                                lhsT,
                                woT_sb[:, kt, n * 512 : (n + 1) * 512],
                                start=(kt == 0),
                                stop=(kt == 15),
                            )
                    out_sb = op.tile([P, E], F32, name="out_sb", tag="outsb")
                    nc.vector.tensor_tensor(
                        out_sb[:MT, :], o_ps[:MT, :], bo_sb[:MT, :], ALU.add
                    )
                    nc.sync.dma_start(out[m * MT : (m + 1) * MT, :], out_sb[:MT, :])

    fixed = _fix_bir_waits(nc.to_json_bytes())
    nc.to_json_bytes = lambda: fixed
    return nc


_NC_CACHE: dict = {}


def _get_nc(S: int) -> bass.Bass:
    if S not in _NC_CACHE:
        _NC_CACHE[S] = build(S)
    return _NC_CACHE[S]


def kernel(
    query,
    key,
    value,
    mask,
    Wq,
    bq,
    Wk,
    bk,
    Wv,
    bv,
    Wo,
    bo,
    _trace: bool = False,
    _trace_dir: str | None = None,
):
    query = np.asarray(query, np.float32)
    key = np.asarray(key, np.float32)
    value = np.asarray(value, np.float32)
    mask = np.ascontiguousarray(np.asarray(mask, np.int32))
    Wq = np.asarray(Wq, np.float32)
    Wk = np.asarray(Wk, np.float32)
    Wv = np.asarray(Wv, np.float32)
    Wo = np.asarray(Wo, np.float32)
    bq = np.asarray(bq, np.float32)
    bk = np.asarray(bk, np.float32)
    bv = np.asarray(bv, np.float32)
    bo = np.asarray(bo, np.float32)

    B, S, E_ = query.shape
    assert (B, E_) == (2, 1024), (B, E_)
    nc = _get_nc(S)

    # host-side layout marshalling (no arithmetic)
    xT = {}
    for g in range(2):
        xT[("q", g)] = np.ascontiguousarray(query[g].T)
        xT[("k", g)] = np.ascontiguousarray(key[g].T)
        xT[("v", g)] = np.ascontiguousarray(value[g].T)
    maskTt = [np.ascontiguousarray(mask[g].T) for g in range(2)]
    WoT = np.ascontiguousarray(Wo.T)
    zeros_w = np.zeros_like(WoT)
    WoTz_g = [
        np.ascontiguousarray(np.concatenate([WoT, zeros_w], axis=0)),
        np.ascontiguousarray(np.concatenate([zeros_w, WoT], axis=0)),
    ]

    in_maps = []
    for c in range(8):
        g, r = divmod(c, 4)
        hs = slice(r * EC, (r + 1) * EC)
        in_maps.append(
            {
                "xqT": xT[("q", g)],
                "xkT": xT[("k", g)],
                "xvT": xT[("v", g)],
                "maskT": maskTt[g],
                "WqT": np.ascontiguousarray(Wq[hs, :].T),
                "WkT": np.ascontiguousarray(Wk[hs, :].T),
                "WvT": np.ascontiguousarray(Wv[hs, :].T),
                "WoTz": WoTz_g[g],
                "bq": np.ascontiguousarray(bq[hs]),
                "bk": np.ascontiguousarray(bk[hs]),
                "bv_b": np.ascontiguousarray(np.broadcast_to(bv[hs], (128, EC))),
                "bo_b": np.ascontiguousarray(np.broadcast_to(bo, (128, 1024))),
            }
        )

    kw = {}
    if _trace:
        kw = dict(trace=True, tmpdir=_trace_dir)
    res = bass_utils.run_bass_kernel_spmd(nc, in_maps, list(range(8)), **kw)

    QBW = S // 4
    out_full = np.empty((B, S, E_), np.float32)
    for c in range(8):
        g, r = divmod(c, 4)
        out_full[g, r * QBW : (r + 1) * QBW, :] = res.results[c]["out"]
    if _trace:
        kernel._last_exec_time_ns = res.exec_time_ns
    return out_full


# revision 20
# speedup vs baseline: 1.2355x; 1.2355x over previous
"""MultiHeadAttention kernel for 8 trn2 NeuronCores (Bass/Tile).

Problem: B=2, S=2048, E=1024, H=16, D=64 (fp32), boolean mask [B,S,S].
  out = softmax(mask((q W_q^T) (k W_k^T)^T / sqrt(D))) (v W_v^T) W_o^T + b_o

Sharding: batch x head-group. Core c (c = 4*g + r) handles batch g and heads
4r..4r+3. Per core:
  - QKV projections for its 4 heads (fp16 matmuls, fp32 PSUM accumulate)
  - attention in transposed layout (scores.T = [k_tok, q_tok]): PE QK with
    2-head row packing, ACT exp straight out of PSUM, DVE mask multiply
    (fp16, 2x mode), PE AV (2-head column packing) + broadcast-rowsum
    matmuls (all-ones stationary)
  - after each q-block: 4-rank AllGather (within the batch group) reshards
    head-rows -> token-slices; all but the last overlap with compute
  - O-projection for this core's 512-token slice; the AllGather output to
    use is selected with a cc_rank-based dynamic DMA offset
Host side does pure layout marshalling (transpose/slice/broadcast/concat).
"""

import sys

sys.path.insert(0, "/opt/trn_rl_repo")

import numpy as np
import concourse.bass as bass
import concourse.mybir as mybir
from concourse.tile import TileContext
from concourse import bass_utils

F32 = mybir.dt.float32
F16 = mybir.dt.float16
I32 = mybir.dt.int32
AF = mybir.ActivationFunctionType
ALU = mybir.AluOpType

P = 128
E = 1024
HPC = 4  # heads per core
EC = HPC * 64  # e_out columns per core (256)
GROUPS = [[0, 1, 2, 3], [4, 5, 6, 7]]

# walrus limits sync-wait commands per instruction (fp32-class matmuls: 1).
# Split excess waits onto NoOps inserted just before, same engine.
_wait_counter = [0]


def _fix_bir_waits(raw: bytes) -> bytes:
    import orjson

    m = orjson.loads(raw)
    for fn in m["functions"]:
        for blk in fn["blocks"]:
            out = []
            changed = False
            for inst in blk["instructions"]:
                si = inst.get("sync_info") or {}
                waits = si.get("on_wait") or []
                if len(waits) > 1:
                    for w in waits[:-1]:
                        _wait_counter[0] += 1
                        out.append(
                            {
                                "engine": inst["engine"],
                                "ins": [],
                                "name": f"I-waitfix-{_wait_counter[0]}",
                                "opcode": "NoOp",
                                "outs": [],
                                "sync_info": {"on_update": [], "on_wait": [w]},
                            }
                        )
                    si["on_wait"] = waits[-1:]
                    inst["sync_info"] = si
                    changed = True
                out.append(inst)
            if changed:
                blk["instructions"] = out
    return orjson.dumps(m)


def build(S: int = 2048) -> bass.Bass:
    KC = S // 128  # k-chunks
    QBW = S // 4  # q-block width = tokens per rank
    NQB = 4
    NW = min(512, QBW)  # attention matmul moving chunk
    NS = min(512, S)  # projection moving chunk
    MT = min(P, QBW)  # output-row tile

    nc = bass.Bass()

    xqT = nc.declare_dram_parameter("xqT", [E, S], F32, isOutput=False)
    xkT = nc.declare_dram_parameter("xkT", [E, S], F32, isOutput=False)
    xvT = nc.declare_dram_parameter("xvT", [E, S], F32, isOutput=False)
    maskT = nc.declare_dram_parameter("maskT", [S, S], I32, isOutput=False)
    WqT = nc.declare_dram_parameter("WqT", [E, EC], F32, isOutput=False)
    WkT = nc.declare_dram_parameter("WkT", [E, EC], F32, isOutput=False)
    WvT = nc.declare_dram_parameter("WvT", [E, EC], F32, isOutput=False)
    WoT = nc.declare_dram_parameter("WoT", [E, E], F32, isOutput=False)
    bq = nc.declare_dram_parameter("bq", [EC], F32, isOutput=False)
    bk = nc.declare_dram_parameter("bk", [EC], F32, isOutput=False)
    bv_b = nc.declare_dram_parameter("bv_b", [P, EC], F32, isOutput=False)
    bo_b = nc.declare_dram_parameter("bo_b", [P, E], F32, isOutput=False)
    out = nc.declare_dram_parameter("out", [QBW, E], F32, isOutput=True)

    with TileContext(nc) as tc:
        with (
            tc.tile_pool(name="persist", bufs=1) as pp,
            tc.tile_pool(name="dramp", bufs=1, space="DRAM") as dramp,
        ):
            ag_in = dramp.tile([NQB, 2 * P, QBW], F16)
            ag_out = dramp.tile([NQB * 4 * 2 * P, QBW], F16)  # [qb][rank][256]

            qT_sb = pp.tile([P, 2, S], F16)  # [:, m, :] = q.T rows 128m..128m+127
            kT_sb = pp.tile([P, 2, S], F16)
            v_sb = pp.tile([P, KC, EC], F16)  # [:, t, :] = v rows 128t..
            ones_sb = pp.tile([P, 64], F16)
            nc.vector.memset(ones_sb[:], 1.0)
            bq_sb = pp.tile([P, 2], F32)
            bk_sb = pp.tile([P, 2], F32)
            nc.sync.dma_start(bq_sb[:], bq.rearrange("(m p) -> p m", p=P))
            nc.sync.dma_start(bk_sb[:], bk.rearrange("(m p) -> p m", p=P))
            bv_sb = pp.tile([P, EC], F16)
            nc.gpsimd.dma_start(bv_sb[:], bv_b[:])
            bo_sb = pp.tile([P, E], F32)
            nc.sync.dma_start(bo_sb[:], bo_b[:])

            # ---------------- Phase A: QKV projections ----------------
            with (
                tc.tile_pool(name="wpool", bufs=1) as wp,
                tc.tile_pool(name="xpool", bufs=3) as xp,
                tc.tile_pool(name="psA", bufs=8, space="PSUM") as psA,
            ):
                wq_sb = wp.tile([P, 8, EC], F16)
                wk_sb = wp.tile([P, 8, EC], F16)
                wv_sb = wp.tile([P, 8, EC], F16)
                nc.gpsimd.dma_start(wq_sb[:], WqT.rearrange("(kt p) m -> p kt m", p=P))
                nc.gpsimd.dma_start(wk_sb[:], WkT.rearrange("(kt p) m -> p kt m", p=P))
                nc.gpsimd.dma_start(wv_sb[:], WvT.rearrange("(kt p) m -> p kt m", p=P))

                for which in range(3):
                    xT, w_sb = [(xqT, wq_sb), (xkT, wk_sb), (xvT, wv_sb)][which]
                    nps = (2 * S) // NS if which < 2 else KC // 2
                    pst = [
                        psA.tile([P, 512], F32, name=f"psA_{which}_{i}", tag="psA")
                        for i in range(nps)
                    ]
                    for kt in range(8):
                        x_t = xp.tile([P, S], F16, name=f"x_{which}_{kt}", tag="x")
                        x_dma = nc.gpsimd.dma_start(x_t[:], xT[kt * P : (kt + 1) * P, :])
                        if which == 2 and kt == 7:
                            last_x_dma = x_dma
                        if which < 2:
                            # q.T / k.T: out [256, S]; lhsT = W tile, rhs = x.T
                            for m in range(2):
                                lhsT = w_sb[:, kt, m * P : (m + 1) * P]
                                for n in range(S // NS):
                                    nc.tensor.matmul(
                                        pst[m * (S // NS) + n][:, :NS],
                                        lhsT,
                                        x_t[:, n * NS : (n + 1) * NS],
                                        start=(kt == 0),
                                        stop=(kt == 7),
                                    )
                        else:
                            # v: out [S, 256]; lhsT = x.T tile, rhs = W k-tile.
                            # Two token-chunks share one PSUM bank: the
                            # has_written group opens on the even chunk and
                            # closes on the odd one (2KB zero-region rule).
                            for t in range(KC):
                                nc.tensor.matmul(
                                    pst[t // 2][:, (t % 2) * EC : (t % 2 + 1) * EC],
                                    x_t[:, t * P : (t + 1) * P],
                                    w_sb[:, kt, :],
                                    start=(kt == 0 and t % 2 == 0),
                                    stop=(kt == 7 and t % 2 == 1),
                                )
                    if which == 0:
                        for m in range(2):
                            for n in range(S // NS):
                                # (q + bq) / 8, bias before scale
                                nc.vector.tensor_scalar(
                                    qT_sb[:, m, n * NS : (n + 1) * NS],
                                    pst[m * (S // NS) + n][:, :NS],
                                    bq_sb[:, m : m + 1],
                                    0.125,
                                    ALU.add,
                                    ALU.mult,
                                )
                    elif which == 1:
                        for m in range(2):
                            for n in range(S // NS):
                                nc.vector.tensor_scalar(
                                    kT_sb[:, m, n * NS : (n + 1) * NS],
                                    pst[m * (S // NS) + n][:, :NS],
                                    1.0,
                                    bk_sb[:, m : m + 1],
                                    ALU.mult,
                                    ALU.add,
                                )
                    else:
                        for t in range(KC):
                            nc.vector.tensor_tensor(
                                v_sb[:, t, :],
                                pst[t // 2][:, (t % 2) * EC : (t % 2 + 1) * EC],
                                bv_sb[:],
                                ALU.add,
                            )

            # ---------------- Phase B: attention + per-qb AllGather ----------
            with (
                tc.tile_pool(name="maskpool", bufs=1) as mp,
                tc.tile_pool(name="ppool", bufs=3) as ppl,
                tc.tile_pool(name="epool", bufs=2) as ep,
                tc.tile_pool(name="sps", bufs=2, space="PSUM") as sps,
                tc.tile_pool(name="avps", bufs=2, space="PSUM") as avps,
                tc.tile_pool(name="rsps", bufs=2, space="PSUM") as rsps,
            ):
                from concourse.tile_rust import add_dep_helper

                maskbf = mp.tile([P, KC, S], F16)
                for t in range(KC):
                    mdma = nc.gpsimd.dma_start(
                        maskbf[:, t, :], maskT[t * P : (t + 1) * P, :]
                    )
                    if t == 0:
                        # keep the big mask stream off phase A's DMA window:
                        # it has plenty of room to stream during phase B
                        add_dep_helper(
                            mdma.ins,
                            last_x_dma.ins,
                            reason="defer mask load until x loads finish",
                        )

                for qb in range(NQB):
                    qsl = slice(qb * QBW, (qb + 1) * QBW)
                    av_t = [
                        avps.tile([P, 512], F32, name=f"av_{qb}_{pair}", tag="av")
                        for pair in range(2)
                    ]
                    rs_t = [
                        rsps.tile([P, 512], F32, name=f"rs_{qb}_{pair}", tag="rs")
                        for pair in range(2)
                    ]
                    for kc in range(KC):
                        ksl = slice(kc * P, (kc + 1) * P)
                        for pair in range(2):
                            s_t = sps.tile(
                                [P, 2, 512], F32, name=f"s_{qb}_{kc}_{pair}", tag="s"
                            )
                            for h in range(2):
                                prt = slice(h * 64, (h + 1) * 64)
                                for n in range(QBW // NW):
                                    nc.tensor.matmul(
                                        s_t[:, h, n * NW : (n + 1) * NW],
                                        kT_sb[prt, pair, ksl],
                                        qT_sb[
                                            prt,
                                            pair,
                                            qb * QBW + n * NW : qb * QBW + (n + 1) * NW,
                                        ],
                                        start=True,
                                        stop=True,
                                    )
                            p_t = ppl.tile([P, 2 * QBW], F16, name="p_t", tag="p")
                            nc.scalar.activation(
                                p_t[:].rearrange("p (h n) -> p h n", h=2),
                                s_t[:, :, :QBW],
                                AF.Exp,
                            )
                            nc.vector.tensor_tensor(
                                p_t[:].rearrange("p (h n) -> p h n", h=2),
                                p_t[:].rearrange("p (h n) -> p h n", h=2),
                                maskbf[:, kc, qsl][:, None, :].to_broadcast(
                                    (P, 2, QBW)
                                ),
                                ALU.mult,
                            )
                            for h in range(2):
                                hsl = slice(h * QBW, (h + 1) * QBW)
                                dsl = slice(pair * P + h * 64, pair * P + (h + 1) * 64)
                                nc.tensor.matmul(
                                    av_t[pair][h * 64 : (h + 1) * 64, :QBW],
                                    v_sb[:, kc, dsl],
                                    p_t[:, hsl],
                                    start=(kc == 0),
                                    stop=(kc == KC - 1),
                                    skip_group_check=(h == 1),
                                )
                                # all-ones stationary -> every output row is
                                # the softmax denominator (broadcast rowsum)
                                nc.tensor.matmul(
                                    rs_t[pair][h * 64 : (h + 1) * 64, :QBW],
                                    ones_sb[:],
                                    p_t[:, hsl],
                                    start=(kc == 0),
                                    stop=(kc == KC - 1),
                                    skip_group_check=(h == 1),
                                )
                    # epilogue for this q-block: divide + stage + AllGather
                    for pair in range(2):
                        rb = ep.tile([P, QBW], F32, name="rb", tag="rb")
                        nc.vector.reciprocal(rb[:], rs_t[pair][:, :QBW])
                        av_f = ep.tile([P, QBW], F16, name="av_f", tag="av_f")
                        nc.vector.tensor_mul(av_f[:], av_t[pair][:, :QBW], rb[:])
                        nc.sync.dma_start(
                            ag_in[qb, pair * P : (pair + 1) * P, :], av_f[:]
                        )
                    nc.gpsimd.collective_compute(
                        "AllGather",
                        ALU.bypass,
                        ins=[ag_in[qb]],
                        outs=[ag_out[qb * 4 * 2 * P : (qb + 1) * 4 * 2 * P, :]],
                        replica_groups=GROUPS,
                    )

            # ---------------- Phase C: O-projection ----------------
            with (
                tc.tile_pool(name="cpool", bufs=1) as cp,
                tc.tile_pool(name="opool", bufs=2) as op,
                tc.tile_pool(name="ops", bufs=2, space="PSUM") as ops,
            ):
                from concourse.tile_rust import add_dep_helper as _adh

                woT_sb = cp.tile([P, 8, E], F16)
                wo_dma = nc.gpsimd.dma_start(
                    woT_sb[:], WoT.rearrange("(kt p) n -> p kt n", p=P)
                )
                _adh(
                    wo_dma.ins,
                    last_x_dma.ins,
                    reason="defer WoT load off phase A's DMA window",
                )
                # my token-slice = AllGather #rank's output
                rank = nc.gpsimd.cc_rank(replica_groups=GROUPS)
                attnT = cp.tile([P, 8, QBW], F16)
                for kt in range(8):
                    nc.gpsimd.dma_start(
                        attnT[:, kt, :],
                        ag_out[bass.ds(rank * (4 * 2 * P) + kt * P, P), :],
                    )
                for m in range(QBW // MT):
                    o_ps = ops.tile([P, E], F32, name=f"o_{m}", tag="o")
                    for kt in range(8):
                        lhsT = attnT[:, kt, m * MT : (m + 1) * MT]
                        for n in range(2):
                            nc.tensor.matmul(
                                o_ps[:MT, n * 512 : (n + 1) * 512],
                                lhsT,
                                woT_sb[:, kt, n * 512 : (n + 1) * 512],
                                start=(kt == 0),
                                stop=(kt == 7),
                            )
                    out_sb = op.tile([P, E], F32, name="out_sb", tag="outsb")
                    nc.vector.tensor_tensor(
                        out_sb[:MT, :], o_ps[:MT, :], bo_sb[:MT, :], ALU.add
                    )
                    nc.sync.dma_start(out[m * MT : (m + 1) * MT, :], out_sb[:MT, :])

    fixed = _fix_bir_waits(nc.to_json_bytes())
    nc.to_json_bytes = lambda: fixed
    return nc


_NC_CACHE: dict = {}


def _get_nc(S: int) -> bass.Bass:
    if S not in _NC_CACHE:
        _NC_CACHE[S] = build(S)
    return _NC_CACHE[S]


def kernel(
    query,
    key,
    value,
    mask,
    Wq,
    bq,
    Wk,
    bk,
    Wv,
    bv,
    Wo,
    bo,
    _trace: bool = False,
    _trace_dir: str | None = None,
):
    query = np.asarray(query, np.float32)
    key = np.asarray(key, np.float32)
    value = np.asarray(value, np.float32)
    mask = np.ascontiguousarray(np.asarray(mask, np.int32))
    Wq = np.asarray(Wq, np.float32)
    Wk = np.asarray(Wk, np.float32)
    Wv = np.asarray(Wv, np.float32)
    Wo = np.asarray(Wo, np.float32)
    bq = np.asarray(bq, np.float32)
    bk = np.asarray(bk, np.float32)
    bv = np.asarray(bv, np.float32)
    bo = np.asarray(bo, np.float32)

    B, S, E_ = query.shape
    assert (B, E_) == (2, 1024), (B, E_)
    nc = _get_nc(S)

    # host-side layout marshalling (no arithmetic)
    xT = {}
    for g in range(2):
        xT[("q", g)] = np.ascontiguousarray(query[g].T)
        xT[("k", g)] = np.ascontiguousarray(key[g].T)
        xT[("v", g)] = np.ascontiguousarray(value[g].T)
    maskTt = [np.ascontiguousarray(mask[g].T) for g in range(2)]
    WoT_h = np.ascontiguousarray(Wo.T)
    bo_rep = np.ascontiguousarray(np.broadcast_to(bo, (128, 1024)))

    in_maps = []
    for c in range(8):
        g, r = divmod(c, 4)
        hs = slice(r * EC, (r + 1) * EC)
        in_maps.append(
            {
                "xqT": xT[("q", g)],
                "xkT": xT[("k", g)],
                "xvT": xT[("v", g)],
                "maskT": maskTt[g],
                "WqT": np.ascontiguousarray(Wq[hs, :].T),
                "WkT": np.ascontiguousarray(Wk[hs, :].T),
                "WvT": np.ascontiguousarray(Wv[hs, :].T),
                "WoT": WoT_h,
                "bq": np.ascontiguousarray(bq[hs]),
                "bk": np.ascontiguousarray(bk[hs]),
                "bv_b": np.ascontiguousarray(np.broadcast_to(bv[hs], (128, EC))),
                "bo_b": bo_rep,
            }
        )

    kw = {}
    if _trace:
        kw = dict(trace=True, tmpdir=_trace_dir)
    res = bass_utils.run_bass_kernel_spmd(nc, in_maps, list(range(8)), **kw)

    QBW = S // 4
    out_full = np.empty((B, S, E_), np.float32)
    for c in range(8):
        g, r = divmod(c, 4)
        out_full[g, r * QBW : (r + 1) * QBW, :] = res.results[c]["out"]
    if _trace:
        kernel._last_exec_time_ns = res.exec_time_ns
        kernel._last_trace = res.instructions_and_trace
    return out_full
